# revision 1
# baseline (speedup 1.0000x reference)
"""Trainium2 Bass kernel for nn_CSRNetGuided: 6-layer hypernetwork-guided
dynamic conv stack (per-sample 3x3 dilated convs + batch BN + ReLU), 1x1 head,
8x nearest upsample.

Sharding: data-parallel over batch (1 sample per NeuronCore, 8 cores); only the
BatchNorm batch statistics are all-reduced across cores.

Self-contained: hardcodes all shapes; imports only environment packages.
"""

import sys
import types

import numpy as np

# ---------------------------------------------------------------------------
# Environment shims (needed before importing concourse.bass_utils):
# the image's antenv lacks axon_hooks; register the NTFF profile hook so
# trace=True works, and keep profile artifacts local (no bucket creds).
# ---------------------------------------------------------------------------
import antenv  # noqa: F401

if "antenv.axon_hooks" not in sys.modules:
    try:
        from trn_agent_boot.trn_boot import _ntff_profile_via_ctypes

        _hook = _ntff_profile_via_ctypes("/opt/axon/libaxon_pjrt.so")
    except Exception:
        _hook = None
    _mod = types.ModuleType("antenv.axon_hooks")
    _mod.get_axon_ntff_profile_hook = lambda: _hook
    _mod.set_axon_ntff_profile_hook = lambda h: None
    sys.modules["antenv.axon_hooks"] = _mod

import bass_rust
import concourse.bacc as bacc
import concourse.bass as bass
import concourse.bass_utils as bass_utils
import concourse.mybir as mybir
import concourse.tile as tile
from concourse.bass_utils import run_bass_kernel_spmd

bass_utils.upload_artifacts = lambda tmpdir: f"local:{tmpdir}"

F32 = mybir.dt.float32
F32R = mybir.dt.float32r
ACTF = mybir.ActivationFunctionType

BIN = [512, 512, 512, 512, 256, 128]
BOUT = [512, 512, 512, 256, 128, 64]
BN_EPS = 1e-5
N_CORES = 8
P = 128
H = W = 64
NSP = 8  # spatial tiles (8 rows each -> 512 output positions)
# tap order: the dy==dx==0 tap first so every accumulation group starts with a
# full-coverage matmul (start=True covers the whole psum tile).
TAP_ORDER = [4, 3, 5, 0, 1, 2, 6, 7, 8]


def _tap_bounds(t, s):
    """Valid output (rows j in tile s, cols w) for tap t; None if empty."""
    dy = 2 * (t // 3 - 1)
    dx = 2 * (t % 3 - 1)
    j_lo = max(0, -(8 * s + dy))
    j_hi = 8 - max(0, (8 * s + 7 + dy) - (H - 1))
    w_lo = max(0, -dx)
    w_hi = W - max(0, dx)
    return dy, dx, j_lo, j_hi, w_lo, w_hi


def _wgen_segments(cin, t, ic):
    """q-variant segments of i-chunk ic for tap t: list of (q, i0, i1)."""
    T3 = 3 * cin
    ilo, ihi = 128 * ic, min(128 * (ic + 1), cin)
    q_lo = (9 * ilo + t) // T3
    q_hi = (9 * (ihi - 1) + t) // T3
    segs = []
    for q in range(q_lo, q_hi + 1):
        i0 = max(ilo, (q * T3 - t + 8) // 9)
        i1 = min(ihi, ((q + 1) * T3 - t + 8) // 9)
        if i0 < i1:
            segs.append((q, i0, i1))
    return segs


def build_kernel(n_layers=6, with_head=True):
    nc = bacc.Bacc(
        "TRN2", target_bir_lowering=False, debug=False, num_devices=N_CORES
    )

    x_ext = nc.declare_dram_parameter("x", [512, H, W], F32, isOutput=False)
    y_ext = nc.declare_dram_parameter("y", [512], F32, isOutput=False)
    fcw_ext = nc.declare_dram_parameter("fc_w", [6, 100, 512], F32, isOutput=False)
    fcb_ext = nc.declare_dram_parameter("fc_b", [6, 100], F32, isOutput=False)
    wb_ext = [
        nc.declare_dram_parameter(f"wb{i}", [3 * BOUT[i], 100], F32, isOutput=False)
        for i in range(6)
    ]
    wc_ext = [
        nc.declare_dram_parameter(f"wc{i}", [3 * BIN[i], 100], F32, isOutput=False)
        for i in range(6)
    ]
    ow_ext = nc.declare_dram_parameter("out_w", [1, 64, 1, 1], F32, isOutput=False)
    ob_ext = nc.declare_dram_parameter("out_b", [1], F32, isOutput=False)
    if n_layers == 6 and with_head:
        out_ext = nc.declare_dram_parameter("out", [8 * H, 8 * W], F32, isOutput=True)
    else:
        cdbg = BOUT[n_layers - 1]
        out_ext = nc.declare_dram_parameter("out", [cdbg, H, W], F32, isOutput=True)

    with tile.TileContext(nc) as tc:
        with (
            tc.tile_pool(name="pers", bufs=1) as pers,
            tc.tile_pool(name="wk", bufs=1) as wk,
            tc.tile_pool(name="ps", bufs=8, space="PSUM") as psp,
            tc.tile_pool(name="dram", bufs=2, space="DRAM") as dram,
        ):
            # ---------------- persistent tiles ----------------
            xa = [pers.tile([P, H, W], F32R, tag=f"xa{c}", name=f"xa{c}") for c in range(4)]
            xb = [pers.tile([P, H, W], F32R, tag=f"xb{c}", name=f"xb{c}") for c in range(4)]
            ident = pers.tile([P, P], F32, tag="ident", name="ident")
            y_sb = pers.tile([P, 4], F32, tag="ysb", name="ysb")
            fcb_sb = pers.tile([100, 6], F32, tag="fcbsb", name="fcbsb")
            wa_all = pers.tile([100, 6], F32, tag="waall", name="waall")
            epsb = pers.tile([P, 1], F32, tag="epsb", name="epsb")
            nc.vector.memset(epsb[:], BN_EPS)
            zsrc = pers.tile([100, P], F32, tag="zsrc", name="zsrc")
            nc.vector.memset(zsrc[:], 0.0)

            nc.gpsimd.memset(ident[:], 0.0)
            nc.gpsimd.affine_select(
                out=ident[:],
                in_=ident[:],
                compare_op=mybir.AluOpType.not_equal,
                fill=1.0,
                base=0,
                pattern=[[-1, P]],
                channel_multiplier=1,
            )

            for c in range(4):
                nc.sync.dma_start(y_sb[:, c : c + 1], y_ext[128 * c : 128 * (c + 1)])
            for l in range(6):
                nc.sync.dma_start(fcb_sb[:, l : l + 1], fcb_ext[l])

            # ---------------- wa for all layers ----------------
            for l in range(n_layers):
                fcw_nat = wk.tile([100, 512], F32, tag="fcwnat", name="fcwnat", bufs=2)
                nc.sync.dma_start(fcw_nat[:], fcw_ext[l])
                wa_ps = psp.tile([P, 512], F32, tag="ps", name="waps")
                for c in range(4):
                    tp = psp.tile([P, 512], F32, tag="ps", name="fcwtp")
                    nc.tensor.transpose(
                        tp[0:P, 0:100],
                        fcw_nat[:, 128 * c : 128 * (c + 1)],
                        ident[0:100, 0:100],
                    )
                    fcwT = wk.tile([P, 100], F32, tag="fcwT", name="fcwT", bufs=2)
                    nc.scalar.activation(fcwT[:], tp[0:P, 0:100], ACTF.Copy)
                    nc.tensor.matmul(
                        wa_ps[0:100, 0:1],
                        fcwT[:],
                        y_sb[:, c : c + 1],
                        start=(c == 0),
                        stop=(c == 3),
                    )
                nc.vector.tensor_add(
                    wa_all[:, l : l + 1], wa_ps[0:100, 0:1], fcb_sb[:, l : l + 1]
                )

            # ------- load x: DMA into (currently idle) wsb slots, round into xa
            for c in range(4):
                xstage = wk.tile([P, 9, 4, P], F32R, tag="wsb", name="xstage", bufs=2)
                flat = xstage[:].rearrange("p a b c -> p (a b c)")[:, 0 : H * W]
                nc.sync.dma_start(
                    flat.bitcast(F32), x_ext[128 * c : 128 * (c + 1)]
                )
                nc.vector.tensor_copy(
                    xa[c][:].rearrange("p a b -> p (a b)"), flat.bitcast(F32)
                )

            # ---------------- layers ----------------
            for l in range(n_layers):
                cin, cout = BIN[l], BOUT[l]
                n_ic = cin // P
                n_oc = (cout + P - 1) // P
                mo = min(P, cout)  # partition extent of output chunks
                src = xa if l % 2 == 0 else xb
                dst = xb if l % 2 == 0 else xa

                # -- per-layer basis transposes: WC_T (f32) and WBwa_T (f32r)
                wct = wk.tile([100, 1536], F32, tag="wct", name="wct")
                wbwat = wk.tile([100, 1536], F32R, tag="wbwat", name="wbwat")
                for c in range(3 * cin // P):
                    nat = wk.tile([P, 100], F32, tag="nat", name="natc", bufs=2)
                    nc.sync.dma_start(nat[:], wc_ext[l][128 * c : 128 * (c + 1)])
                    tp = psp.tile([P, 512], F32, tag="ps", name="wctp")
                    nc.tensor.transpose(tp[0:100, 0:P], nat[:], ident[:])
                    nc.scalar.activation(
                        wct[:, 128 * c : 128 * (c + 1)], tp[0:100, 0:P], ACTF.Copy
                    )
                for c in range((3 * cout + P - 1) // P):
                    rows = min(P, 3 * cout - P * c)
                    nat = wk.tile([P, 100], F32, tag="nat", name="natb", bufs=2)
                    nc.sync.dma_start(
                        nat[0:rows, :], wb_ext[l][128 * c : 128 * c + rows]
                    )
                    tp = psp.tile([P, 512], F32, tag="ps", name="wbtp")
                    nc.tensor.transpose(
                        tp[0:100, 0:rows], nat[0:rows, :], ident[0:rows, 0:rows]
                    )
                    nc.scalar.activation(
                        wbwat[:, 128 * c : 128 * c + rows],
                        tp[0:100, 0:rows],
                        ACTF.Copy,
                        scale=wa_all[:, l : l + 1],
                    )

                # -- weight generation -> DRAM (layout [oc][t][ic][i][o'])
                wdram = dram.tile([4, 9, 4, P, P], F32R, tag="wdram", name="wdram")
                n_half = (cout + 255) // 256
                for t in range(9):
                    segs_by_ic = [_wgen_segments(cin, t, ic) for ic in range(n_ic)]
                    for ic in range(n_ic):
                        segs = segs_by_ic[ic]
                        wcem = wk.tile(
                            [100, 3, P], F32R, tag="wcem", name="wcem", bufs=3
                        )
                        full = len(segs) == 1 and (
                            segs[0][1] == 128 * ic and segs[0][2] == 128 * (ic + 1)
                        )
                        for v, (q, i0, i1) in enumerate(segs):
                            if not full:
                                nc.vector.tensor_copy(wcem[:, v, :], zsrc[:])
                            cnt = i1 - i0
                            s0 = 9 * i0 + t - q * 3 * cin
                            nc.vector.tensor_copy(
                                wcem[:, v, i0 - 128 * ic : i1 - 128 * ic],
                                wct[:, s0 : s0 + 9 * (cnt - 1) + 1 : 9],
                            )
                        wps = psp.tile([P, 512], F32, tag="ps", name="wps")
                        for h in range(n_half):
                            ow = min(256, cout - 256 * h)
                            for v, (q, i0, i1) in enumerate(segs):
                                r0 = 3 * 256 * h + q
                                nc.tensor.matmul(
                                    wps[:, 256 * h : 256 * h + ow],
                                    wcem[:, v, :],
                                    wbwat[:, r0 : r0 + 3 * (ow - 1) + 1 : 3],
                                    start=(v == 0),
                                    stop=(v == len(segs) - 1),
                                )
                        wstage = wk.tile([P, 512], F32R, tag="wstage", name="wstage", bufs=2)
                        nc.scalar.activation(
                            wstage[:, 0:cout], wps[:, 0:cout], ACTF.Copy
                        )
                        nc.sync.dma_start(
                            wdram[:, t, ic, :, :]
                            .rearrange("oc i o -> i oc o")[0:P, 0:n_oc, 0:mo],
                            wstage[:, 0:cout].rearrange("p (oc o) -> p oc o", o=mo),
                        )

                # -- stats accumulators
                spack = wk.tile([P, 4, 2], F32, tag="spack", name="spack", bufs=2)
                nc.vector.memset(spack[:].rearrange("p a b -> p (a b)"), 0.0)

                # -- conv per output chunk (weights prefetched from DRAM)
                for oc in range(n_oc):
                    wsb = wk.tile([P, 9, 4, P], F32R, tag="wsb", name="wsb", bufs=2)
                    for t in range(9):
                        nc.sync.dma_start(
                            wsb[:, t, 0:n_ic, 0:mo],
                            wdram[oc, t, 0:n_ic, :, 0:mo].rearrange(
                                "ic i o -> i ic o"
                            ),
                        )
                    pst = [
                        psp.tile([P, 512], F32, tag="ps", name=f"cps{s}")
                        for s in range(NSP)
                    ]
                    pairs = [(t, ic) for t in TAP_ORDER for ic in range(n_ic)]
                    for pi, (t, ic) in enumerate(pairs):
                        for s in range(NSP):
                            dy, dx, j_lo, j_hi, w_lo, w_hi = _tap_bounds(t, s)
                            rhs = src[ic][
                                :,
                                8 * s + dy + j_lo : 8 * s + dy + j_hi,
                                w_lo + dx : w_hi + dx,
                            ]
                            out_ap = pst[s][:].rearrange("p (a b) -> p a b", a=NSP)[
                                0:mo, j_lo:j_hi, w_lo:w_hi
                            ]
                            nc.tensor.matmul(
                                out_ap,
                                wsb[:, t, ic, 0:mo],
                                rhs,
                                start=(pi == 0),
                                stop=(pi == len(pairs) - 1),
                                skip_group_check=True,
                            )
                    st6 = wk.tile([P, NSP, 6], F32, tag="st6", name="st6", bufs=2)
                    for s in range(NSP):
                        nc.vector.bn_stats(st6[0:mo, s, :], pst[s][0:mo, :])
                        nc.scalar.activation(
                            dst[oc][0:mo, 8 * s : 8 * s + 8, :].rearrange(
                                "p a b -> p (a b)"
                            ),
                            pst[s][0:mo, :],
                            ACTF.Copy,
                        )
                    aggr = wk.tile([P, 2], F32, tag="aggr", name="aggr", bufs=2)
                    nc.vector.bn_aggr(aggr[0:mo, :], st6[0:mo, :, :])
                    nc.vector.tensor_copy(spack[0:mo, oc, 0:1], aggr[0:mo, 0:1])
                    msq = wk.tile([P, 1], F32, tag="msq", name="msq", bufs=2)
                    nc.vector.tensor_mul(
                        msq[0:mo, :], aggr[0:mo, 0:1], aggr[0:mo, 0:1]
                    )
                    nc.vector.tensor_add(
                        spack[0:mo, oc, 1:2], aggr[0:mo, 1:2], msq[0:mo, :]
                    )

                # -- all-reduce batch stats across the 8 cores
                cc_in = dram.tile([P, 8], F32, tag="ccin", name="ccin")
                cc_out = dram.tile([P, 8], F32, tag="ccout", name="ccout")
                nc.sync.dma_start(
                    cc_in[:], spack[:].rearrange("p a b -> p (a b)")
                )
                nc.gpsimd.collective_compute(
                    "AllReduce",
                    mybir.AluOpType.add,
                    replica_groups=[list(range(N_CORES))],
                    ins=[cc_in[:].opt()],
                    outs=[cc_out[:].opt()],
                )
                gst = wk.tile([P, 4, 2], F32, tag="gst", name="gst", bufs=2)
                nc.sync.dma_start(gst[:].rearrange("p a b -> p (a b)"), cc_out[:])
                # finish: mean = g0/8; m2 = g1/8; var = m2 - mean^2
                g8 = wk.tile([P, 4, 2], F32, tag="g8", name="g8", bufs=2)
                nc.vector.tensor_scalar_mul(
                    g8[:].rearrange("p a b -> p (a b)"),
                    gst[:].rearrange("p a b -> p (a b)"),
                    1.0 / N_CORES,
                )
                mean_ap = g8[:, 0:n_oc, 0]
                m2_ap = g8[:, 0:n_oc, 1]
                varb = wk.tile([P, 4], F32, tag="varb", name="varb", bufs=2)
                nc.vector.tensor_mul(varb[:, 0:n_oc], mean_ap, mean_ap)
                nc.vector.tensor_sub(varb[:, 0:n_oc], m2_ap, varb[:, 0:n_oc])
                stdb = wk.tile([P, 4], F32, tag="stdb", name="stdb", bufs=2)
                nc.scalar.activation(
                    stdb[:, 0:n_oc], varb[:, 0:n_oc], ACTF.Sqrt, bias=epsb[:, 0:1]
                )
                invb = wk.tile([P, 4], F32, tag="invb", name="invb", bufs=2)
                nc.vector.reciprocal(invb[:, 0:n_oc], stdb[:, 0:n_oc])
                nbias = wk.tile([P, 4], F32, tag="nbias", name="nbias", bufs=2)
                nc.vector.tensor_mul(nbias[:, 0:n_oc], mean_ap, invb[:, 0:n_oc])
                nc.vector.tensor_scalar_mul(nbias[:, 0:n_oc], nbias[:, 0:n_oc], -1.0)

                # -- normalize + ReLU (in place, writes f32r)
                for oc in range(n_oc):
                    flat = dst[oc][0:mo].rearrange("p a b -> p (a b)")
                    nc.scalar.activation(
                        flat,
                        flat.bitcast(F32),
                        ACTF.Relu,
                        scale=invb[0:mo, oc : oc + 1],
                        bias=nbias[0:mo, oc : oc + 1],
                    )

            # ---------------- head: 1x1 conv + 8x nearest upsample ----------
            fin = xa if n_layers % 2 == 0 else xb
            if n_layers == 6 and with_head:
                ow_sb = wk.tile([64, 1], F32, tag="owsb", name="owsb")
                nc.sync.dma_start(ow_sb[:], ow_ext[:])
                ow_r = wk.tile([64, 1], F32R, tag="owr", name="owr")
                nc.vector.tensor_copy(ow_r[:], ow_sb[:])
                ob_sb = wk.tile([1, 1], F32, tag="obsb", name="obsb")
                nc.sync.dma_start(ob_sb[:], ob_ext[:])
                v_sb = wk.tile([64, 64], F32, tag="vsb", name="vsb")
                for s in range(NSP):
                    hps = psp.tile([P, 512], F32, tag="ps", name="hps")
                    nc.tensor.matmul(
                        hps[0:1, :],
                        ow_r[:],
                        fin[0][0:64, 8 * s : 8 * s + 8, :].rearrange(
                            "p a b -> p (a b)"
                        ),
                        start=True,
                        stop=True,
                    )
                    hstage = wk.tile([1, 512], F32, tag="hstage", name="hstage", bufs=2)
                    nc.vector.tensor_scalar_add(
                        hstage[:], hps[0:1, :], ob_sb[0:1, 0:1]
                    )
                    nc.sync.dma_start(v_sb[8 * s : 8 * s + 8, :], hstage[:])
                wide = wk.tile([64, 8 * W], F32, tag="wide", name="wide")
                for dw in range(8):
                    nc.vector.tensor_copy(
                        wide[:].rearrange("p (w e) -> p w e", e=8)[:, :, dw], v_sb[:]
                    )
                oview = out_ext.rearrange("(h e) w -> h e w", e=8)
                for dh in range(8):
                    nc.sync.dma_start(oview[:, dh, :], wide[:])
            else:
                # debug head: dump post-BN activations of the last built layer
                cdbg = BOUT[n_layers - 1]
                for oc in range((cdbg + P - 1) // P):
                    mo = min(P, cdbg - P * oc)
                    nc.sync.dma_start(
                        out_ext[P * oc : P * oc + mo],
                        fin[oc][0:mo].bitcast(F32).rearrange("p a b -> p (a b)"),
                    )

    nc.compile()
    return nc


_CACHED = {}


def kernel(**inputs):
    x = np.ascontiguousarray(np.asarray(inputs["x"], dtype=np.float32))
    y = np.ascontiguousarray(np.asarray(inputs["y"], dtype=np.float32))
    B = x.shape[0]
    assert B == N_CORES
    shared_keys = (
        ["fc_w", "fc_b"]
        + [f"wb{i}" for i in range(6)]
        + [f"wc{i}" for i in range(6)]
        + ["out_w", "out_b"]
    )
    shared = {
        k: np.ascontiguousarray(np.asarray(inputs[k], dtype=np.float32))
        for k in shared_keys
    }

    if "nc" not in _CACHED:
        _CACHED["nc"] = build_kernel()
    nc = _CACHED["nc"]

    in_maps = [dict(shared, x=x[b], y=y[b]) for b in range(B)]
    res = run_bass_kernel_spmd(nc, in_maps, list(range(N_CORES)))
    out = np.stack([res.results[b]["out"][None] for b in range(B)])
    return out.astype(np.float32)


if __name__ == "__main__":
    import reference as ref

    inputs = ref.setup_inputs()
    got = kernel(**{k: np.asarray(v) for k, v in inputs.items()})
    expected = np.asarray(ref.reference(**inputs))
    err = np.linalg.norm(got - expected) / np.linalg.norm(expected)
    print("shapes:", got.shape, expected.shape)
    print("Relative error:", err)



# revision 2
# speedup vs baseline: 1.1465x; 1.1465x over previous
"""Trainium2 Bass kernel for nn_CSRNetGuided: 6-layer hypernetwork-guided
dynamic conv stack (per-sample 3x3 dilated convs + batch BN + ReLU), 1x1 head,
8x nearest upsample.

Sharding: data-parallel over batch (1 sample per NeuronCore, 8 cores); only the
BatchNorm batch statistics are all-reduced across cores.

v2: bf16 activations + bf16 generated weights held fully in SBUF (no DRAM
round-trip), per-oc-chunk BN collectives pipelined under the conv, weight
generation for layer l+1 interleaved into conv of layer l, ic-major conv
accumulation order so the next layer's conv starts before the previous layer
is fully normalized.

Self-contained: hardcodes all shapes; imports only environment packages.
"""

import sys
import types

import numpy as np

# ---------------------------------------------------------------------------
# Environment shims (needed before importing concourse.bass_utils):
# the image's antenv lacks axon_hooks; register the NTFF profile hook so
# trace=True works, and keep profile artifacts local (no bucket creds).
# ---------------------------------------------------------------------------
import antenv  # noqa: F401

if "antenv.axon_hooks" not in sys.modules:
    try:
        from trn_agent_boot.trn_boot import _ntff_profile_via_ctypes

        _hook = _ntff_profile_via_ctypes("/opt/axon/libaxon_pjrt.so")
    except Exception:
        _hook = None
    _mod = types.ModuleType("antenv.axon_hooks")
    _mod.get_axon_ntff_profile_hook = lambda: _hook
    _mod.set_axon_ntff_profile_hook = lambda h: None
    sys.modules["antenv.axon_hooks"] = _mod

import bass_rust
import concourse.bacc as bacc
import concourse.bass as bass
import concourse.bass_utils as bass_utils
import concourse.mybir as mybir
import concourse.tile as tile
from concourse.bass_utils import run_bass_kernel_spmd

bass_utils.upload_artifacts = lambda tmpdir: f"local:{tmpdir}"

F32 = mybir.dt.float32
BF16 = mybir.dt.bfloat16
ACTF = mybir.ActivationFunctionType

BIN = [512, 512, 512, 512, 256, 128]
BOUT = [512, 512, 512, 256, 128, 64]
BN_EPS = 1e-5
N_CORES = 8
P = 128
H = W = 64
NSP = 8  # spatial tiles (8 rows each -> 512 output positions)
# tap order: the dy==dx==0 tap first so the first accumulation group starts
# with a full-coverage matmul (start=True covers the whole psum tile).
TAP_ORDER = [4, 3, 5, 0, 1, 2, 6, 7, 8]


def _tap_bounds(t, s):
    """Valid output (rows j in tile s, cols w) for tap t; None if empty."""
    dy = 2 * (t // 3 - 1)
    dx = 2 * (t % 3 - 1)
    j_lo = max(0, -(8 * s + dy))
    j_hi = 8 - max(0, (8 * s + 7 + dy) - (H - 1))
    w_lo = max(0, -dx)
    w_hi = W - max(0, dx)
    return dy, dx, j_lo, j_hi, w_lo, w_hi


def _wgen_segments(cin, t, ic):
    """q-variant segments of i-chunk ic for tap t: list of (q, i0, i1)."""
    T3 = 3 * cin
    ilo, ihi = 128 * ic, min(128 * (ic + 1), cin)
    q_lo = (9 * ilo + t) // T3
    q_hi = (9 * (ihi - 1) + t) // T3
    segs = []
    for q in range(q_lo, q_hi + 1):
        i0 = max(ilo, (q * T3 - t + 8) // 9)
        i1 = min(ihi, ((q + 1) * T3 - t + 8) // 9)
        if i0 < i1:
            segs.append((q, i0, i1))
    return segs


def build_kernel(n_layers=6, with_head=True):
    nc = bacc.Bacc(
        "TRN2", target_bir_lowering=False, debug=False, num_devices=N_CORES
    )

    x_ext = nc.declare_dram_parameter("x", [512, H, W], F32, isOutput=False)
    y_ext = nc.declare_dram_parameter("y", [512], F32, isOutput=False)
    fcw_ext = nc.declare_dram_parameter("fc_w", [6, 100, 512], F32, isOutput=False)
    fcb_ext = nc.declare_dram_parameter("fc_b", [6, 100], F32, isOutput=False)
    wb_ext = [
        nc.declare_dram_parameter(f"wb{i}", [3 * BOUT[i], 100], F32, isOutput=False)
        for i in range(6)
    ]
    wc_ext = [
        nc.declare_dram_parameter(f"wc{i}", [3 * BIN[i], 100], F32, isOutput=False)
        for i in range(6)
    ]
    ow_ext = nc.declare_dram_parameter("out_w", [1, 64, 1, 1], F32, isOutput=False)
    ob_ext = nc.declare_dram_parameter("out_b", [1], F32, isOutput=False)
    if n_layers == 6 and with_head:
        out_ext = nc.declare_dram_parameter("out", [8 * H, 8 * W], F32, isOutput=True)
    else:
        cdbg = BOUT[n_layers - 1]
        out_ext = nc.declare_dram_parameter("out", [cdbg, H, W], F32, isOutput=True)

    with tile.TileContext(nc) as tc:
        with (
            tc.tile_pool(name="pers", bufs=1) as pers,
            tc.tile_pool(name="wk", bufs=1) as wk,
            tc.tile_pool(name="ps", bufs=8, space="PSUM") as psp,
            tc.tile_pool(name="dram", bufs=2, space="DRAM") as dram,
        ):
            # ---------------- persistent tiles ----------------
            xa = [pers.tile([P, H, W], BF16, tag=f"xa{c}", name=f"xa{c}") for c in range(4)]
            xb = [pers.tile([P, H, W], BF16, tag=f"xb{c}", name=f"xb{c}") for c in range(4)]
            # ping-pong full-layer weight store: [i, tap, ic, cout]
            wbuf = [
                pers.tile([P, 9, 4, 512], BF16, tag=f"wl{p}", name=f"wl{p}")
                for p in range(2)
            ]
            ident = pers.tile([P, P], F32, tag="ident", name="ident")
            y_sb = pers.tile([P, 4], F32, tag="ysb", name="ysb")
            fcb_sb = pers.tile([100, 6], F32, tag="fcbsb", name="fcbsb")
            wa_all = pers.tile([100, 6], F32, tag="waall", name="waall")
            epsb = pers.tile([P, 1], F32, tag="epsb", name="epsb")
            nc.vector.memset(epsb[:], BN_EPS)
            zsrc = pers.tile([100, P], BF16, tag="zsrc", name="zsrc")
            nc.vector.memset(zsrc[:], 0.0)

            nc.gpsimd.memset(ident[:], 0.0)
            nc.gpsimd.affine_select(
                out=ident[:],
                in_=ident[:],
                compare_op=mybir.AluOpType.not_equal,
                fill=1.0,
                base=0,
                pattern=[[-1, P]],
                channel_multiplier=1,
            )

            for c in range(4):
                nc.sync.dma_start(y_sb[:, c : c + 1], y_ext[128 * c : 128 * (c + 1)])
            for l in range(6):
                nc.sync.dma_start(fcb_sb[:, l : l + 1], fcb_ext[l])

            # ---------------- wa for all layers ----------------
            for l in range(n_layers):
                fcw_nat = wk.tile([100, 512], F32, tag="fcwnat", name="fcwnat", bufs=2)
                nc.sync.dma_start(fcw_nat[:], fcw_ext[l])
                wa_ps = psp.tile([P, 512], F32, tag="ps", name="waps")
                for c in range(4):
                    tp = psp.tile([P, 512], F32, tag="ps", name="fcwtp")
                    nc.tensor.transpose(
                        tp[0:P, 0:100],
                        fcw_nat[:, 128 * c : 128 * (c + 1)],
                        ident[0:100, 0:100],
                    )
                    fcwT = wk.tile([P, 100], F32, tag="fcwT", name="fcwT", bufs=2)
                    nc.scalar.activation(fcwT[:], tp[0:P, 0:100], ACTF.Copy)
                    nc.tensor.matmul(
                        wa_ps[0:100, 0:1],
                        fcwT[:],
                        y_sb[:, c : c + 1],
                        start=(c == 0),
                        stop=(c == 3),
                    )
                nc.vector.tensor_add(
                    wa_all[:, l : l + 1], wa_ps[0:100, 0:1], fcb_sb[:, l : l + 1]
                )

            # ------- load x and convert to bf16 (alternate scalar/vector)
            for c in range(4):
                for h2 in range(2):
                    xst = wk.tile([P, 32, W], F32, tag="xst", name="xst", bufs=2)
                    nc.sync.dma_start(
                        xst[:], x_ext[128 * c : 128 * (c + 1)][:, 32 * h2 : 32 * h2 + 32, :]
                    )
                    dst_ap = xa[c][:, 32 * h2 : 32 * h2 + 32, :].rearrange(
                        "p a b -> p (a b)"
                    )
                    src_ap = xst[:].rearrange("p a b -> p (a b)")
                    if h2 == 0:
                        nc.scalar.activation(dst_ap, src_ap, ACTF.Copy)
                    else:
                        nc.vector.tensor_copy(dst_ap, src_ap)

            # ---------------- weight generation ----------------
            # basis: wct = wc^T (bf16), wbwat = (wb*wa)^T (bf16); single buffers
            wct = pers.tile([100, 1536], BF16, tag="wct", name="wct")
            wbwat = pers.tile([100, 1536], BF16, tag="wbwat", name="wbwat")

            def wgen_basis(l):
                cin, cout = BIN[l], BOUT[l]
                for c in range(3 * cin // P):
                    nat = wk.tile([P, 100], F32, tag="nat", name="natc", bufs=2)
                    nc.sync.dma_start(nat[:], wc_ext[l][128 * c : 128 * (c + 1)])
                    tp = psp.tile([P, 512], F32, tag="ps", name="wctp")
                    nc.tensor.transpose(tp[0:100, 0:P], nat[:], ident[:])
                    nc.scalar.activation(
                        wct[:, 128 * c : 128 * (c + 1)], tp[0:100, 0:P], ACTF.Copy
                    )
                for c in range((3 * cout + P - 1) // P):
                    rows = min(P, 3 * cout - P * c)
                    nat = wk.tile([P, 100], F32, tag="nat", name="natb", bufs=2)
                    nc.sync.dma_start(
                        nat[0:rows, :], wb_ext[l][128 * c : 128 * c + rows]
                    )
                    tp = psp.tile([P, 512], F32, tag="ps", name="wbtp")
                    nc.tensor.transpose(
                        tp[0:100, 0:rows], nat[0:rows, :], ident[0:rows, 0:rows]
                    )
                    nc.scalar.activation(
                        wbwat[:, 128 * c : 128 * c + rows],
                        tp[0:100, 0:rows],
                        ACTF.Copy,
                        scale=wa_all[:, l : l + 1],
                    )

            def wgen_mm(l):
                """Generate layer-l weights into wbuf[l % 2] (bf16, in SBUF)."""
                cin, cout = BIN[l], BOUT[l]
                n_ic = cin // P
                wdst = wbuf[l % 2]
                for t in range(9):
                    for ic in range(n_ic):
                        segs = _wgen_segments(cin, t, ic)
                        wcem = wk.tile(
                            [100, 3, P], BF16, tag="wcem", name="wcem", bufs=3
                        )
                        full = len(segs) == 1 and (
                            segs[0][1] == 128 * ic and segs[0][2] == 128 * (ic + 1)
                        )
                        for v, (q, i0, i1) in enumerate(segs):
                            if not full:
                                nc.vector.tensor_copy(wcem[:, v, :], zsrc[:])
                            cnt = i1 - i0
                            s0 = 9 * i0 + t - q * 3 * cin
                            nc.vector.tensor_copy(
                                wcem[:, v, i0 - 128 * ic : i1 - 128 * ic],
                                wct[:, s0 : s0 + 9 * (cnt - 1) + 1 : 9],
                            )
                        wps = psp.tile([P, 512], F32, tag="ps", name="wps")
                        for v, (q, i0, i1) in enumerate(segs):
                            nc.tensor.matmul(
                                wps[:, 0:cout],
                                wcem[:, v, :],
                                wbwat[:, q : q + 3 * (cout - 1) + 1 : 3],
                                start=(v == 0),
                                stop=(v == len(segs) - 1),
                            )
                        nc.scalar.activation(
                            wdst[:, t, ic, 0:cout], wps[0:P, 0:cout], ACTF.Copy
                        )

            wgen_basis(0)
            wgen_mm(0)
            if n_layers > 1:
                wgen_basis(1)

            # ---------------- per-layer conv machinery ----------------
            def conv_chunk(l, oc, src, dst, spk):
                """One output-channel chunk: 2 waves of 4 spatial tiles each.
                Writes raw conv output (bf16) to dst[oc], per-wave stats into
                spk[:, 2w:2w+2] = (mean, E[x^2])."""
                cin, cout = BIN[l], BOUT[l]
                n_ic = cin // P
                mo = min(P, cout - P * oc)
                pairs = [(ic, t) for ic in range(n_ic) for t in TAP_ORDER]
                st6 = wk.tile([P, NSP, 6], F32, tag="st6", name="st6", bufs=2)
                for w in range(2):
                    ss = list(range(4 * w, 4 * w + 4))
                    pst = {
                        s: psp.tile([P, 512], F32, tag="ps", name=f"cps{s}")
                        for s in ss
                    }
                    for pi, (ic, t) in enumerate(pairs):
                        lhs = wbuf[l % 2][:, t, ic, P * oc : P * oc + mo]
                        for s in ss:
                            dy, dx, j_lo, j_hi, w_lo, w_hi = _tap_bounds(t, s)
                            rhs = src[ic][
                                :,
                                8 * s + dy + j_lo : 8 * s + dy + j_hi,
                                w_lo + dx : w_hi + dx,
                            ]
                            out_ap = pst[s][:].rearrange(
                                "p (a b) -> p a b", a=NSP
                            )[0:mo, j_lo:j_hi, w_lo:w_hi]
                            nc.tensor.matmul(
                                out_ap,
                                lhs,
                                rhs,
                                start=(pi == 0),
                                stop=(pi == len(pairs) - 1),
                                skip_group_check=True,
                            )
                    for s in ss:
                        nc.vector.bn_stats(st6[0:mo, s, :], pst[s][0:mo, :])
                        nc.scalar.activation(
                            dst[oc][0:mo, 8 * s : 8 * s + 8, :].rearrange(
                                "p a b -> p (a b)"
                            ),
                            pst[s][0:mo, :],
                            ACTF.Copy,
                        )
                    aggr = wk.tile([P, 2], F32, tag="aggr", name="aggr", bufs=2)
                    nc.vector.bn_aggr(aggr[0:mo, :], st6[0:mo, 4 * w : 4 * w + 4, :])
                    nc.vector.tensor_copy(spk[0:mo, 2 * w : 2 * w + 1], aggr[0:mo, 0:1])
                    msq = wk.tile([P, 1], F32, tag="msq", name="msq", bufs=2)
                    nc.vector.tensor_mul(
                        msq[0:mo, :], aggr[0:mo, 0:1], aggr[0:mo, 0:1]
                    )
                    nc.vector.tensor_add(
                        spk[0:mo, 2 * w + 1 : 2 * w + 2], aggr[0:mo, 1:2], msq[0:mo, :]
                    )

            def issue_cc(spk):
                """All-reduce one chunk's per-wave stats; returns SBUF result."""
                cc_in = dram.tile([P, 4], F32, tag="ccin", name="ccin", bufs=2)
                cc_out = dram.tile([P, 4], F32, tag="ccout", name="ccout", bufs=2)
                nc.sync.dma_start(cc_in[:], spk[:])
                nc.gpsimd.collective_compute(
                    "AllReduce",
                    mybir.AluOpType.add,
                    replica_groups=[list(range(N_CORES))],
                    ins=[cc_in[:].opt()],
                    outs=[cc_out[:].opt()],
                )
                gst = wk.tile([P, 4], F32, tag="gst", name="gst", bufs=3)
                nc.sync.dma_start(gst[:], cc_out[:])
                return gst

            def norm_chunk(l, oc, dst, gst):
                cout = BOUT[l]
                mo = min(P, cout - P * oc)
                # mean = (sumA_mean + sumB_mean)/16 ; m2 likewise
                g8 = wk.tile([P, 4], F32, tag="g8", name="g8", bufs=2)
                nc.vector.tensor_scalar_mul(g8[:], gst[:], 1.0 / (2 * N_CORES))
                meanb = wk.tile([P, 1], F32, tag="meanb", name="meanb", bufs=2)
                nc.vector.tensor_add(meanb[:], g8[:, 0:1], g8[:, 2:3])
                m2b = wk.tile([P, 1], F32, tag="m2b", name="m2b", bufs=2)
                nc.vector.tensor_add(m2b[:], g8[:, 1:2], g8[:, 3:4])
                varb = wk.tile([P, 1], F32, tag="varb", name="varb", bufs=2)
                nc.vector.tensor_mul(varb[:], meanb[:], meanb[:])
                nc.vector.tensor_sub(varb[:], m2b[:], varb[:])
                stdb = wk.tile([P, 1], F32, tag="stdb", name="stdb", bufs=2)
                nc.scalar.activation(
                    stdb[:], varb[:], ACTF.Sqrt, bias=epsb[:, 0:1]
                )
                invb = wk.tile([P, 1], F32, tag="invb", name="invb", bufs=2)
                nc.vector.reciprocal(invb[:], stdb[:])
                nbias = wk.tile([P, 1], F32, tag="nbias", name="nbias", bufs=2)
                nc.vector.tensor_mul(nbias[:], meanb[:], invb[:])
                nc.vector.tensor_scalar_mul(nbias[:], nbias[:], -1.0)
                flat = dst[oc][0:mo].rearrange("p a b -> p (a b)")
                nc.scalar.activation(
                    flat,
                    flat,
                    ACTF.Relu,
                    scale=invb[0:mo, 0:1],
                    bias=nbias[0:mo, 0:1],
                )

            # ---------------- layers ----------------
            for l in range(n_layers):
                cout = BOUT[l]
                n_oc = (cout + P - 1) // P
                src = xa if l % 2 == 0 else xb
                dst = xb if l % 2 == 0 else xa
                gsts = [None] * n_oc
                for oc in range(n_oc):
                    spk = wk.tile([P, 4], F32, tag="spk", name="spk", bufs=2)
                    conv_chunk(l, oc, src, dst, spk)
                    gsts[oc] = issue_cc(spk)
                    if oc == 0 and l + 1 < n_layers:
                        wgen_mm(l + 1)
                        if l + 2 < n_layers:
                            wgen_basis(l + 2)
                    if oc >= 1:
                        norm_chunk(l, oc - 1, dst, gsts[oc - 1])
                norm_chunk(l, n_oc - 1, dst, gsts[n_oc - 1])

            # ---------------- head: 1x1 conv + 8x nearest upsample ----------
            fin = xa if n_layers % 2 == 0 else xb
            if n_layers == 6 and with_head:
                ow_sb = wk.tile([64, 1], F32, tag="owsb", name="owsb")
                nc.sync.dma_start(ow_sb[:], ow_ext[:])
                ow_r = wk.tile([64, 1], BF16, tag="owr", name="owr")
                nc.vector.tensor_copy(ow_r[:], ow_sb[:])
                ob_sb = wk.tile([1, 1], F32, tag="obsb", name="obsb")
                nc.sync.dma_start(ob_sb[:], ob_ext[:])
                v_sb = wk.tile([64, 64], F32, tag="vsb", name="vsb")
                for s in range(NSP):
                    hps = psp.tile([P, 512], F32, tag="ps", name="hps")
                    nc.tensor.matmul(
                        hps[0:1, :],
                        ow_r[:],
                        fin[0][0:64, 8 * s : 8 * s + 8, :].rearrange(
                            "p a b -> p (a b)"
                        ),
                        start=True,
                        stop=True,
                    )
                    hstage = wk.tile([1, 512], F32, tag="hstage", name="hstage", bufs=2)
                    nc.vector.tensor_scalar_add(
                        hstage[:], hps[0:1, :], ob_sb[0:1, 0:1]
                    )
                    nc.sync.dma_start(v_sb[8 * s : 8 * s + 8, :], hstage[:])
                wide = wk.tile([64, 8 * W], F32, tag="wide", name="wide")
                for dw in range(8):
                    nc.vector.tensor_copy(
                        wide[:].rearrange("p (w e) -> p w e", e=8)[:, :, dw], v_sb[:]
                    )
                oview = out_ext.rearrange("(h e) w -> h e w", e=8)
                for dh in range(8):
                    nc.sync.dma_start(oview[:, dh, :], wide[:])
            else:
                # debug head: dump post-BN activations of the last built layer
                cdbg = BOUT[n_layers - 1]
                for oc in range((cdbg + P - 1) // P):
                    mo = min(P, cdbg - P * oc)
                    dbg = wk.tile([P, H * W], F32, tag="dbg", name="dbg", bufs=2)
                    nc.scalar.activation(
                        dbg[0:mo, :],
                        fin[oc][0:mo].rearrange("p a b -> p (a b)"),
                        ACTF.Copy,
                    )
                    nc.sync.dma_start(
                        out_ext[P * oc : P * oc + mo], dbg[0:mo, :]
                    )

    nc.compile()
    return nc


_CACHED = {}


def kernel(**inputs):
    x = np.ascontiguousarray(np.asarray(inputs["x"], dtype=np.float32))
    y = np.ascontiguousarray(np.asarray(inputs["y"], dtype=np.float32))
    B = x.shape[0]
    assert B == N_CORES
    shared_keys = (
        ["fc_w", "fc_b"]
        + [f"wb{i}" for i in range(6)]
        + [f"wc{i}" for i in range(6)]
        + ["out_w", "out_b"]
    )
    shared = {
        k: np.ascontiguousarray(np.asarray(inputs[k], dtype=np.float32))
        for k in shared_keys
    }

    if "nc" not in _CACHED:
        _CACHED["nc"] = build_kernel()
    nc = _CACHED["nc"]

    in_maps = [dict(shared, x=x[b], y=y[b]) for b in range(B)]
    res = run_bass_kernel_spmd(nc, in_maps, list(range(N_CORES)))
    out = np.stack([res.results[b]["out"][None] for b in range(B)])
    return out.astype(np.float32)


if __name__ == "__main__":
    import reference as ref

    inputs = ref.setup_inputs()
    got = kernel(**{k: np.asarray(v) for k, v in inputs.items()})
    expected = np.asarray(ref.reference(**inputs))
    err = np.linalg.norm(got - expected) / np.linalg.norm(expected)
    print("shapes:", got.shape, expected.shape)
    print("Relative error:", err)
